# revision 53
# baseline (speedup 1.0000x reference)
"""Trainium2 Bass kernel for nn_Attention_42674795053784.

Full cross-attention block: q/kv projections, per-head RMSNorm + RoPE on q/k,
softmax(q k^T / sqrt(d)) @ v, output projection.

Sharding: 8 cores = 4 batches x 2 head-groups (tensor parallel over heads,
data parallel over batch). Each core computes a partial [n, DIM] output
(its 8 heads' contribution through its Wo row-slice); host sums core pairs.

Device dataflow per core (flipped-PV design):
  Projections bf16, k-tiled over the 1024 contraction; RMSNorm rsqrt batched
  via masked-ones matmuls (ssq, allocated late to keep the pq PSUM slots
  free) + Ln/Exp on ScalarE; rstd broadcast via a hmT rank-2 matmul.
  scores^T [m, n] per head fp32 -> exp on ScalarE (scale=1/8) -> bf16 probs.
  PV is FLIPPED: xa[n-tile 128, 65] += pexp_slice^T @ v_aug, so the matmul
  free dim is 65 instead of 512 (PE time scales with out free size; PV drops
  from 109us to 56us). xa slices are padded to 128 floats so no matmul output
  crosses a PSUM bank, and the first PV matmul of each hp round uses
  start=True per bank (zero-region arming) while all others accumulate with
  start=False. Normalize = DVE reciprocal of the den column + stride-0
  broadcast multiply; PE transposes (bf16, via identity) put x back into
  [d, n] layout for the bf16 output projection.
  Scheduling: chunk-0 scores/exp for hp0/hp1 are emitted interleaved with the
  kv chunks that unblock them (pexp pool buffers 28 tiles) so ScalarE fills
  during the kv phase; qproj for chunk j+1 is emitted at hp0 of chunk j with
  high priority; act/table DMAs ride the Activation HWDGE queue, weights ride
  SP's, and the startup-critical wk/wq/act0 loads are split for latency.
"""
import os
import numpy as np

_K = lambda name, dflt: int(os.environ.get(name, dflt))
P_SC = _K("P_SC", 4500)    # scores+exp priority offset (positive=earlier)
P_PV = _K("P_PV", 3000)       # PV matmul priority
P_V = _K("P_V", 1500)  # v_mm priority
P_TR = _K("P_TR", 2000)    # transpose priority
P_OP = _K("P_OP", -6000)   # outproj priority
P_QP = _K("P_QP", 1000)    # qproj priority
P_K = _K("P_K", 0)         # k-side projection priority
QP_AT = _K("QP_AT", 0)     # emit qproj after this hp

B, N, M, DIM = 4, 2048, 2048, 1024
H, D = 16, 64
HPC = 8            # heads per core
EPC = HPC * D      # 512 output dims per core
NCH = 512          # n/m chunk size
NCHUNKS = N // NCH
KT = DIM // 128    # 8 k-tiles over dim
PT = EPC // 128    # 4 pair-tiles (2 heads each)
MT = M // 128      # 16 m-tiles
EPS = float(np.finfo(np.float32).eps)
ROPE_THETA = 10000.0

_CACHE = {}


def _build_nc():
    import concourse.bacc as bacc
    import concourse.tile as tile
    import concourse.mybir as mybir

    F32 = mybir.dt.float32
    BF16 = mybir.dt.bfloat16
    AF = mybir.ActivationFunctionType
    ALU = mybir.AluOpType

    import bass_rust as _bass_rust
    from concourse.hw_specs import get_activation_tables

    class _OneSetBacc(bacc.Bacc):
        # Constrain activation-table choice to the single set containing both
        # Ln and Exp so the fixpoint inserts exactly one ACT_TABLE_LOAD.
        def insert_act_table_loads(self):
            has_activation = any(
                isinstance(i, mybir.InstActivation)
                for b in self.main_func.blocks
                for i in b.instructions
            )
            if not has_activation:
                return
            tables = [(k, v if k == "natural_log_exp_and_others" else set())
                      for k, v in get_activation_tables(self.m.arch).items()]
            _bass_rust.insert_act_table_loads(self, tables)

    nc = _OneSetBacc("TRN2", target_bir_lowering=False)

    # activations / weights: k-tiled [128, k, n]
    tgt8_d = nc.dram_tensor("tgt8", [128, KT, N], BF16, kind="ExternalInput")
    src8_d = nc.dram_tensor("src8", [128, KT, M], BF16, kind="ExternalInput")
    wq_d = nc.dram_tensor("wq", [128, KT, EPC], BF16, kind="ExternalInput")
    wk_d = nc.dram_tensor("wk", [128, KT, EPC], BF16, kind="ExternalInput")
    wv_d = nc.dram_tensor("wv", [128, KT, EPC], BF16, kind="ExternalInput")
    wo_d = nc.dram_tensor("wo", [128, PT, DIM], BF16, kind="ExternalInput")
    csq_d = nc.dram_tensor("csq", [128, 2, N], BF16, kind="ExternalInput")
    csk_d = nc.dram_tensor("csk", [128, 2, M], BF16, kind="ExternalInput")
    hm_d = nc.dram_tensor("hm32", [128, 32], BF16, kind="ExternalInput")
    hmT_d = nc.dram_tensor("hmT128", [128, 128], BF16, kind="ExternalInput")
    onc_d = nc.dram_tensor("onc", [128, 8], BF16, kind="ExternalInput")
    eye_d = nc.dram_tensor("eye128", [128, 128], BF16, kind="ExternalInput")
    eps_d = nc.dram_tensor("epsb", [128, 1], F32, kind="ExternalInput")
    out_d = nc.dram_tensor("out", [N, DIM], BF16, kind="ExternalOutput")

    from contextlib import ExitStack
    with ExitStack() as _es:
        tc = _es.enter_context(tile.TileContext(nc))
        _p = lambda **kw: _es.enter_context(tc.tile_pool(**kw))
        cst = _p(name="cst", bufs=1)
        wt = _p(name="wt", bufs=3)
        actp = _p(name="actp", bufs=3)
        tabp = _p(name="tabp", bufs=4)
        prjp = _p(name="prjp", bufs=5)
        sqp = _p(name="sqp", bufs=2)
        cbp = _p(name="cbp", bufs=3)
        rsp = _p(name="rsp", bufs=2)
        rbp = _p(name="rbp", bufs=4)
        ktp = _p(name="ktp", bufs=4)
        qtp = _p(name="qtp", bufs=4)
        vap = _p(name="vap", bufs=16)
        xtp = _p(name="xtp", bufs=8)
        xnp = _p(name="xnp", bufs=2)
        ppp = _p(name="ppp", bufs=2)
        nrm = _p(name="nrm", bufs=3)
        obp = _p(name="obp", bufs=4)
        # PSUM (8 banks): pq tag (prj/vps/ssq/ops/pstr rotation) 2 banks;
        # psc (sc2 double-buffer) 4 banks; xa 1 bank; den 1 bank.
        pq = _p(name="pq", bufs=2, space="PSUM")
        psc = _p(name="psc", bufs=2, space="PSUM")
        pX = _p(name="pX", bufs=1, space="PSUM")
        if True:
            # ---- weights first (k proj is the critical path at start);
            # split the first loads so k-proj p0 can start ~5us earlier ----
            wk_t = wt.tile([128, KT, EPC], BF16, name="wk", tag="wt")
            nc.sync.dma_start(out=wk_t[:, :, 0:128], in_=wk_d[:, :, 0:128])
            nc.sync.dma_start(out=wk_t[:, :, 128:EPC], in_=wk_d[:, :, 128:EPC])
            act0 = actp.tile([128, KT, NCH], BF16, name="actk0", tag="act")
            for _k in range(0, KT, 2):
                nc.scalar.dma_start(out=act0[:, _k:_k + 2, :], in_=src8_d[:, _k:_k + 2, 0:NCH])
            wv_t = wt.tile([128, KT, EPC], BF16, name="wv", tag="wt")

            # ---- constants ----
            hm32 = cst.tile([128, 32], BF16, name="hm32", tag="hm")
            nc.sync.dma_start(out=hm32, in_=hm_d[:, :])
            hmT = cst.tile([128, 128], BF16, name="hmT", tag="hmT")
            nc.sync.dma_start(out=hmT, in_=hmT_d[:, :])
            epsb = cst.tile([128, 1], F32, name="epsb", tag="epsb")
            nc.sync.dma_start(out=epsb, in_=eps_d[:, :])
            onc = cst.tile([128, 8], BF16, name="onc", tag="onc")
            nc.sync.dma_start(out=onc, in_=onc_d[:, :])
            eye = cst.tile([128, 128], BF16, name="eye", tag="eye")
            nc.sync.dma_start(out=eye, in_=eye_d[:, :])

            kt_t = [ktp.tile([128, M], BF16, name=f"kt{p}", tag="kt") for p in range(PT)]
            qt_tiles = {}  # (p, chunk) -> [128, NCH] tile

            def qt_tile(p, j):
                if (p, j) not in qt_tiles:
                    qt_tiles[(p, j)] = qtp.tile([128, NCH], BF16, name=f"qt{p}_{j}", tag="qt", bufs=8)
                return qt_tiles[(p, j)]
            va_t = []  # [128, 8, 65] bf16 per m-tile

            def proj_mm(prj, w_t, act, p):
                """prj [128, NCH] PSUM = (w pair-slice)^T @ act."""
                for k in range(KT):
                    nc.tensor.matmul(prj, w_t[:, k, p * 128:(p + 1) * 128],
                                     act[:, k, :],
                                     start=(k == 0), stop=(k == KT - 1))

            def v_mm(vps, act, b):
                """vps [128, EPC] PSUM = act m-block^T @ wv."""
                for k in range(KT):
                    nc.tensor.matmul(vps, act[:, k, b * 128:(b + 1) * 128],
                                     wv_t[:, k, :],
                                     start=(k == 0), stop=(k == KT - 1))

            def proj_chunk(pref, j, w_t, act, cs_sb, dst):
                """All 4 pair-tiles of one chunk: proj + RMSNorm + RoPE."""
                kside = pref == "k"
                prjs_l = []
                sq_l = []
                for p in range(PT):
                    prj = pq.tile([128, NCH], F32, name=f"prj{pref}{j}_{p}", tag="pq")
                    proj_mm(prj, w_t, act, p)
                    prjs = prjp.tile([128, NCH], BF16, name=f"prjs{pref}{j}_{p}", tag="prjs")
                    if kside:
                        nc.scalar.copy(prjs, prj)
                    else:
                        nc.vector.tensor_copy(prjs, prj)
                    prjs_l.append(prjs)
                    sq = sqp.tile([128, NCH], BF16, name=f"sq{pref}{j}_{p}", tag="sq", bufs=5)
                    nc.vector.tensor_mul(sq, prjs, prjs)
                    sq_l.append(sq)
                # ssq allocated late: pins a pq slot only for the 4 MMs + Ln
                ssq = pq.tile([128, NCH], F32, name=f"ssq{pref}{j}", tag="pq")
                for p in range(PT):
                    nc.tensor.matmul(ssq[32 * p:32 * p + 32, :], hm32, sq_l[p],
                                     start=True, stop=True, skip_group_check=True,
                                     tile_position=(0, 32 * p))
                lnv = nrm.tile([128, NCH], F32, name=f"lnv{pref}{j}", tag="lnv", bufs=2)
                nc.scalar.activation(lnv, ssq, AF.Ln, scale=1.0 / 64.0, bias=epsb)
                rstd = rsp.tile([128, NCH], BF16, name=f"rstd{pref}{j}", tag="rstd")
                nc.scalar.activation(rstd, lnv, AF.Exp, scale=-0.5)
                for p in range(PT):
                    # broadcast rstd rows (32p -> partitions 0:64, 32p+1 -> 64:128)
                    rb = pq.tile([128, NCH], F32, name=f"rb{pref}{j}_{p}", tag="pq")
                    nc.tensor.matmul(rb, hmT[32 * p:32 * p + 32, :], rstd[32 * p:32 * p + 32, :],
                                     start=True, stop=True, skip_group_check=True,
                                     tile_position=(32 * p, 0))
                    prjs = prjs_l[p]
                    ca = cbp.tile([128, NCH], BF16, name="ca", tag="ca", bufs=2)
                    nc.vector.tensor_mul(ca, prjs, cs_sb[:, 0, :])
                    cb = cbp.tile([128, NCH], BF16, name="cb", tag="cb")
                    for qd in range(4):
                        sig = qd + 1 if qd % 2 == 0 else qd - 1
                        eng = nc.gpsimd if (kside and qd >= 2) else nc.vector
                        eng.tensor_mul(cb[qd * 32:(qd + 1) * 32, :],
                                       prjs[sig * 32:(sig + 1) * 32, :],
                                       cs_sb[sig * 32:(sig + 1) * 32, 1, :])
                    nc.vector.tensor_add(cb, cb, ca)
                    nc.vector.tensor_mul(dst(p, j), cb, rb)

            # ---- phase B: K/V projections over m-chunks ----
            def kv_chunk(j):
                if j == 0:
                    act = act0
                else:
                    act = actp.tile([128, KT, NCH], BF16, name=f"actk{j}", tag="act")
                    nc.scalar.dma_start(out=act, in_=src8_d[:, :, j * NCH:(j + 1) * NCH])
                cs_sb = tabp.tile([128, 2, NCH], BF16, name=f"csk{j}", tag="tab")
                nc.scalar.dma_start(out=cs_sb, in_=csk_d[:, :, j * NCH:(j + 1) * NCH])
                with tc.high_priority(offset=P_K):
                    proj_chunk("k", j, wk_t, act, cs_sb,
                               lambda p_, j_: kt_t[p_][:, j_ * NCH:(j_ + 1) * NCH])
                for b in range(4):
                    mt = j * 4 + b
                    vps = pq.tile([128, NCH], F32, name=f"vps{mt}", tag="pq")
                    with tc.high_priority(offset=P_V):
                        v_mm(vps, act, b)
                    va = vap.tile([128, HPC, 65], BF16, name=f"va{mt}", tag="va")
                    nc.vector.tensor_copy(va[:, :, 0:64],
                                          vps.rearrange("p (h e) -> p h e", h=HPC))
                    nc.gpsimd.tensor_copy(va[:, :, 64:65],
                                          onc.rearrange("p (h e) -> p h e", e=1))
                    va_t.append(va)

            wq_t = wt.tile([128, KT, EPC], BF16, name="wq", tag="wt")
            nc.sync.dma_start(out=wq_t[:, :, 0:128], in_=wq_d[:, :, 0:128])
            nc.sync.dma_start(out=wq_t[:, :, 128:EPC], in_=wq_d[:, :, 128:EPC])
            nc.sync.dma_start(out=wv_t, in_=wv_d[:, :, :])

            def q_loads(j):
                act = actp.tile([128, KT, NCH], BF16, name=f"actq{j}", tag="act")
                for _k in range(0, KT, 2):
                    nc.scalar.dma_start(out=act[:, _k:_k + 2, :],
                                        in_=tgt8_d[:, _k:_k + 2, j * NCH:(j + 1) * NCH])
                cs_sb = tabp.tile([128, 2, NCH], BF16, name=f"csq{j}", tag="tab")
                nc.scalar.dma_start(out=cs_sb, in_=csq_d[:, :, j * NCH:(j + 1) * NCH])
                return act, cs_sb

            kv_chunk(0)
            q0 = q_loads(0)
            proj_chunk("q", 0, wq_t, q0[0], q0[1], lambda p_, j_: qt_tile(p_, j_))

            # ---- Wo (one DMA) ----
            wo_t = wt.tile([128, PT, DIM], BF16, name="wo", tag="wo", bufs=1)
            nc.sync.dma_start(out=wo_t, in_=wo_d[:, :, :])

            # ---- phase D: attention + output projection per n-chunk ----
            def outproj(j, xts):
                for t in range(4):
                    osb = obp.tile([128, DIM], BF16, name=f"osb{j}_{t}", tag="osb")
                    for ob in range(2):
                        ops = pq.tile([128, NCH], F32, name=f"ops{j}_{t}_{ob}", tag="pq")
                        for p in range(PT):
                            nc.tensor.matmul(ops, xts[p][:, t * 128:(t + 1) * 128],
                                             wo_t[:, p, ob * NCH:(ob + 1) * NCH],
                                             start=(p == 0), stop=(p == PT - 1))
                        nc.vector.tensor_copy(osb[:, ob * NCH:(ob + 1) * NCH], ops)
                    nc.sync.dma_start(out=out_d[j * NCH + t * 128: j * NCH + (t + 1) * 128, :],
                                      in_=osb)

            def attn_scores(j, hp, g):
                """Scores + exp for m-tiles 2g, 2g+1 -> [pexp(par0), pexp(par1)]."""
                pexps = []
                for par in range(2):
                    lo, hi = par * 64, par * 64 + 64
                    sc = psc.tile([128, 2 * NCH], F32, name=f"sc{j}_{hp}_{g}_{par}", tag="sc")
                    with tc.high_priority(offset=P_SC):
                        for u in range(2):
                            i = g * 2 + u
                            nc.tensor.matmul(sc[:, u * NCH:(u + 1) * NCH],
                                             kt_t[hp][lo:hi, i * 128:(i + 1) * 128],
                                             qt_tile(hp, j)[lo:hi, :],
                                             start=True, stop=True, skip_group_check=True)
                        pexp = ppp.tile([128, 2 * NCH], BF16, name="pexp", tag="pexp", bufs=28)
                        nc.scalar.activation(pexp, sc, AF.Exp, scale=0.125)
                    pexps.append(pexp)
                return pexps

            def attn_pv(j, hp, g, xa, pexps):
                for par in range(2):
                    pexp = pexps[par]
                    h = 2 * hp + par
                    with tc.high_priority(offset=P_PV):
                        for u in range(2):
                            i = g * 2 + u
                            for nt in range(4):
                                lhsT = pexp[:, u * NCH + nt * 128: u * NCH + (nt + 1) * 128]
                                first = (g == 0 and u == 0 and nt == 0)  # arms this par's bank
                                last = (g == MT // 2 - 1 and u == 1 and nt == 3)
                                s = par * 4 + nt
                                nc.tensor.matmul(xa[:, s, 0:65], lhsT, va_t[i][:, h, :],
                                                 start=first, stop=last, skip_group_check=True)

            def attn_group(j, hp, g, xa, pre=None):
                attn_pv(j, hp, g, xa, pre if pre is not None else attn_scores(j, hp, g))

            def attn_finish(j, hp, xa, xts):
                """Normalize by den, transpose back to [d, n], copy to xts."""
                xn2 = xnp.tile([128, 4, 128], BF16, name=f"xn{j}_{hp}", tag="xn")
                rden = nrm.tile([128, 8], F32, name=f"rden{j}_{hp}", tag="rden", bufs=2)
                nc.vector.reciprocal(rden, xa[:, :, 64])

                for par in range(2):
                    den_b = rden[:, par * 4:(par + 1) * 4].unsqueeze(-1).broadcast_to([128, 4, 64])
                    nc.vector.tensor_tensor(xn2[:, :, par * 64:(par + 1) * 64],
                                            xa[:, par * 4:(par + 1) * 4, 0:64], den_b, ALU.mult)
                pstr = pq.tile([128, 4, 128], BF16, name=f"pstr{j}_{hp}", tag="pq")
                with tc.high_priority(offset=P_TR):
                    for nt in range(4):
                        nc.tensor.transpose(pstr[:, nt, :], xn2[:, nt, :], eye)
                nc.vector.tensor_copy(xts[hp], pstr.rearrange("p a b -> p (a b)"))

            pending = None
            for j in range(NCHUNKS):
                qnext = q_loads(j + 1) if j + 1 < NCHUNKS else None
                xts = [None] * PT
                pre01 = {}
                if j == 0:
                    # kv-paced interleave: emit each k chunk, then the hp0/hp1
                    # scores+exp it unblocks, keeping ScalarE fed during phase B;
                    # v projections trail one chunk behind the k they share acts
                    # with (PV consumption lags via the pexp buffer).
                    for gp in range(MT // 4):
                        if gp + 1 < NCHUNKS:
                            kv_chunk(gp + 1)
                        for hp in range(2):
                            for g in (2 * gp, 2 * gp + 1):
                                pre01[(hp, g)] = attn_scores(j, hp, g)

                for hp in range(PT):
                    xts[hp] = xtp.tile([128, NCH], BF16, name=f"xt{j}_{hp}", tag="xt")
                    xa = pX.tile([128, 8, 128], F32, name=f"xa{j}_{hp}", tag="xa")
                    for g in range(MT // 2):
                        attn_group(j, hp, g, xa, pre=pre01.get((hp, g)))
                    attn_finish(j, hp, xa, xts)
                    if hp == 0 and pending is not None:
                        with tc.high_priority(offset=P_OP):
                            outproj(*pending)
                        pending = None
                    if qnext is not None and hp == QP_AT:
                        with tc.high_priority(offset=P_QP):
                            proj_chunk("q", j + 1, wq_t, qnext[0], qnext[1],
                                       lambda p_, j_: qt_tile(p_, j_))
                pending = (j, xts)
            outproj(*pending)
    nc.finalize()
    return nc


def _host_prep(tgt, src, tgt_pos, src_pos, Wq, Wkv, Wo, q_norm_w, k_norm_w):
    """Build the 8 per-core input maps."""
    import ml_dtypes
    f32 = np.float32
    bf16 = ml_dtypes.bfloat16
    inv_freq = (1.0 / (ROPE_THETA ** (np.arange(0, D, 2, dtype=f32) / f32(D)))).astype(f32)

    def pair_pack(a, dt):
        # [1024, n] -> [128, k, n], plain k-tiles d = 128k + p
        n = a.shape[1]
        r = a.reshape(KT, 128, n).transpose(1, 0, 2)
        return np.ascontiguousarray(r).astype(dt)

    def tables(pos, w):
        # pos [n] int32, w [64] -> [128, 2, n] bf16 (cos ; sign-folded sin)
        ang = pos.astype(f32)[:, None] * inv_freq[None, :]          # [n, 32]
        c = np.cos(ang).astype(f32)
        s = np.sin(ang).astype(f32)
        C = np.empty((64, pos.shape[0]), f32)
        C[0:32] = (c * w[0:32][None, :]).T
        C[32:64] = (c * w[32:64][None, :]).T
        S = np.empty((64, pos.shape[0]), f32)
        S[0:32] = (s * w[0:32][None, :]).T
        S[32:64] = -(s * w[32:64][None, :]).T
        cs = np.stack([np.concatenate([C, C], 0), np.concatenate([S, S], 0)], axis=1)
        return np.ascontiguousarray(cs).astype(bf16)

    hm32 = np.zeros((128, 32), f32)
    hm32[0:64, 0] = 1.0
    hm32[64:128, 1] = 1.0
    hmT = np.zeros((128, 128), f32)
    for p in range(4):
        hmT[32 * p + 0, 0:64] = 1.0
        hmT[32 * p + 1, 64:128] = 1.0
    consts = {
        "hm32": hm32.astype(bf16), "hmT128": hmT.astype(bf16),
        "onc": np.ones((128, 8), f32).astype(bf16),
        "eye128": np.eye(128, dtype=f32).astype(bf16),
        "epsb": np.full((128, 1), EPS, f32),
    }

    in_maps = []
    Wk_full, Wv_full = Wkv[:, 0:DIM], Wkv[:, DIM:2 * DIM]
    for bi in range(B):
        tgt8 = pair_pack(np.ascontiguousarray(tgt[bi].T), bf16)
        src8 = pair_pack(np.ascontiguousarray(src[bi].T), bf16)
        csq = tables(tgt_pos[bi], np.asarray(q_norm_w, f32))
        csk = tables(src_pos[bi], np.asarray(k_norm_w, f32))
        for g in range(2):
            cols = slice(g * EPC, (g + 1) * EPC)
            wo_g = np.ascontiguousarray(Wo[cols, :]).reshape(PT, 128, DIM)
            in_maps.append({
                "tgt8": tgt8, "src8": src8,
                "wq": pair_pack(np.ascontiguousarray(Wq[:, cols]), bf16),
                "wk": pair_pack(np.ascontiguousarray(Wk_full[:, cols]), bf16),
                "wv": pair_pack(np.ascontiguousarray(Wv_full[:, cols]), bf16),
                "wo": np.ascontiguousarray(wo_g.transpose(1, 0, 2)).astype(bf16),
                "csq": csq, "csk": csk,
                **consts,
            })
    return in_maps


def kernel(tgt, src, tgt_pos, src_pos, Wq, Wkv, Wo, q_norm_w, k_norm_w, **kw):
    from concourse.bass_utils import run_bass_kernel_spmd

    tgt = np.asarray(tgt, np.float32)
    src = np.asarray(src, np.float32)
    Wq = np.asarray(Wq, np.float32)
    Wkv = np.asarray(Wkv, np.float32)
    Wo = np.asarray(Wo, np.float32)

    if "nc" not in _CACHE:
        _CACHE["nc"] = _build_nc()
    nc = _CACHE["nc"]

    in_maps = _host_prep(tgt, src, tgt_pos, src_pos, Wq, Wkv, Wo, q_norm_w, k_norm_w)
    res = run_bass_kernel_spmd(nc, in_maps, core_ids=list(range(8)), **kw)
    _CACHE["last_results"] = res
    parts = [np.asarray(r["out"], np.float32) for r in res.results]
    out = np.stack([parts[2 * bi] + parts[2 * bi + 1] for bi in range(B)])
    return out.astype(np.float32)
